# revision 4
# baseline (speedup 1.0000x reference)
"""Trainium2 Bass kernel for nn_MoEsparseRoutingForClassification.

Reference computation (B=64, S=128, H=1024, E=8, L=2):
    x = X[:, 0, :]                                   # CLS token [B,H]
    y[b,o]   = sum_e g[b,e] * (x[b] . dense_w[e,o,:]) + (g @ dense_b)[b,o]
    t        = tanh(y)
    out[b,l] = sum_e g[b,e] * (t[b] . out_w[e,l,:])  + (g @ out_b)[b,l]

Distribution: the H output dim of the dense layer is sharded 8 ways
(OC=128 per core).  Core c computes y[:, c*OC:(c+1)*OC] (which needs the
full CLS token but only a slice dense_w[:, c_slice, :]), applies tanh,
and contracts its slice against out_w[:, :, c_slice] to produce a
partial [L,128] logit block.  The partials (incl. the out_b bias, fed
only to core 0) sum to the full output on the host.  No cross-core
collective is needed.

The stage-1 stream (w1 weights + CLS activations + dense_b bias) is
carried in bf16: it halves the dominant HBM traffic (4.3 -> 2.2 MiB per
core) and runs the PE in single-pass bf16 mode instead of dual-pass
fp32 LOW_HIGH.  Stage 2 and the gate math stay fp32.  rel-err budget is
2e-2; bf16 rounding lands ~5e-3.

DMA ring layout (per-descriptor issue cost ~0.7-1.5us on the issuing
engine, doorbell->data ~1-2us, dma-complete->sem-visible ~0.5us):
  sync ring:   w1 in 3 chunks (3/3/2 k-tiles) so the PE chases chunk
               completions; nothing else queues ahead of it.
  gpsimd ring: xt (the ring is otherwise idle; overlaps the w1 train).
  scalar ring: dbp FIRST (the PE bias fold waits on it), then ep.
               gc is gone entirely: the gate-broadcast table is built
               by a K=8 PE matmul from gtdup/ebc consts in ep.

On-core layout: OC=128 is split into two 64-wide halves mapped to PSUM
partition halves (rows 64h+b).  dense_b rides into the same PSUM
accumulation via a K=1 matmul (ones x db) placed before the k-loop, so
the post-mix DVE chain is just mult + 3 tree adds + tanh.  The final
[128,2] result is PE-transposed into a [2,128] PSUM tile pre-seeded
with the gate-mixed out_b (K=8 matmul), so the output DMA is 2 rows x
512 B instead of 128 rows x 8 B.

Everything arithmetic runs on device; the host only slices, transposes
(layout prep), and sums the partial outputs.
"""

import sys

import numpy as np

for _p in ("/opt/trn_rl_repo",):
    if _p not in sys.path:
        sys.path.insert(0, _p)

# If the environment sets BASS_TRACE but lacks antenv.axon_hooks (this agent
# image does), run_bass_kernel_spmd would crash on import; pre-seed a no-op
# module so tracing degrades gracefully instead.
try:  # pragma: no cover
    import antenv.axon_hooks  # noqa: F401
except Exception:  # pragma: no cover
    import types as _types

    _m = _types.ModuleType("antenv.axon_hooks")
    _m._hook = None
    _m.set_axon_ntff_profile_hook = lambda h: setattr(_m, "_hook", h)
    _m.get_axon_ntff_profile_hook = lambda: _m._hook
    sys.modules["antenv.axon_hooks"] = _m

B, S, H = 64, 128, 1024
E, L = 8, 2
NCORES = 8
OC = H // NCORES          # dense-output slice per core (128)
HC = OC // 2              # half-slice mapped to a PSUM partition half (64)
KT = H // 128             # contraction tiles
P = 128

W1_CHUNKS = ((0, 3), (3, 6), (6, 8))

_cached = None


def _build():
    from contextlib import ExitStack

    import concourse.tile as tile
    from concourse import bacc, mybir

    F32 = mybir.dt.float32
    BF16 = mybir.dt.bfloat16
    AF = mybir.ActivationFunctionType
    OP = mybir.AluOpType

    nc = bacc.Bacc("TRN2", target_bir_lowering=False, debug=False,
                   num_devices=NCORES)

    # E-pack along the free dim (one DMA for all E-partition consts):
    #   gt [E,B] | ow2 [E,2,L,HC] | ob [E,L] | gtz [E,P] | gtdup [E,P]
    #   | ebc [E,E*HC]
    EPACK = B + L * OC + L + P + P + E * HC
    xt_d = nc.dram_tensor("xt", [P, KT, B], BF16, kind="ExternalInput")
    w1_d = nc.dram_tensor("w1", [P, KT, 2, E, HC], BF16, kind="ExternalInput")
    db_d = nc.dram_tensor("dbp", [1, 2 * E * HC], BF16, kind="ExternalInput")
    ep_d = nc.dram_tensor("ep", [E, EPACK], F32, kind="ExternalInput")
    out_d = nc.dram_tensor("out", [L, P], F32, kind="ExternalOutput")

    with tile.TileContext(nc) as tc, ExitStack() as ctx:
        consts = ctx.enter_context(tc.tile_pool(name="consts", bufs=1))
        wpool = ctx.enter_context(tc.tile_pool(name="wpool", bufs=1))
        mixp = ctx.enter_context(tc.tile_pool(name="mixp", bufs=2))
        smallp = ctx.enter_context(tc.tile_pool(name="smallp", bufs=1))
        psy = ctx.enter_context(tc.tile_pool(name="psy", bufs=1, space="PSUM"))
        pss = ctx.enter_context(tc.tile_pool(name="pss", bufs=1, space="PSUM"))
        psg = ctx.enter_context(tc.tile_pool(name="psg", bufs=1, space="PSUM"))

        w1_t = wpool.tile([P, KT, 2, E, HC], BF16)
        for klo, khi in W1_CHUNKS:
            nc.sync.dma_start(
                out=w1_t[:, klo:khi],
                in_=w1_d.ap()[:, klo:khi],
            )
        xt_t = consts.tile([P, KT, B], BF16)
        nc.gpsimd.dma_start(out=xt_t, in_=xt_d.ap())
        db_t = consts.tile([1, 2 * E * HC], BF16)
        nc.scalar.dma_start(out=db_t, in_=db_d.ap())
        ep_t = consts.tile([E, EPACK], F32)
        nc.scalar.dma_start(out=ep_t, in_=ep_d.ap())
        o = 0
        gt_t = ep_t[:, o:o + B]; o += B
        ow_t = ep_t[:, o:o + L * OC].rearrange(
            "e (h l c) -> e h l c", h=2, l=L); o += L * OC
        ob_t = ep_t[:, o:o + L]; o += L
        gtz_t = ep_t[:, o:o + P]; o += P         # gates.T | zeros
        gtdup_t = ep_t[:, o:o + P]; o += P       # gates.T | gates.T
        ebc_t = ep_t[:, o:o + E * HC]            # kron(I_E, ones[HC])

        # ---- small matmuls first so their consumers unblock early ----
        # sel_ow^h [64h+b, (l, hc)]
        psum_ow = pss.tile([P, L, HC], F32)
        for h in range(2):
            sl = slice(h * 64, h * 64 + 64)
            nc.tensor.matmul(
                psum_ow[sl, :, :].rearrange("b l c -> b (l c)"),
                gt_t, ow_t[:, h].rearrange("e l c -> e (l c)"),
                start=True, stop=True, skip_group_check=True,
            )
        # Output accumulator [l, p]: seed with sel_ob^T = ob^T-mixed gates
        # (only core 0 carries real ob), then the stage-2 transpose
        # accumulates on top.
        psum_o2 = pss.tile([L, P], F32)
        nc.tensor.matmul(psum_o2[:], ob_t, gtz_t,
                         start=True, stop=False, skip_group_check=True)
        # Gate-broadcast table gb[p, (e, hc)] = g[b, e] via one K=8 matmul
        # (kills the slow 128-row gc DMA and 8 DVE builds).
        psum_gb = psg.tile([P, E, HC], F32)
        nc.tensor.matmul(psum_gb[:, :, :].rearrange("p e c -> p (e c)"),
                         gtdup_t, ebc_t, start=True, stop=True)
        gb_t = consts.tile([P, E, HC], F32)
        nc.vector.tensor_copy(gb_t[:], psum_gb[:])

        # ---- dense_b fold: psum_y[64h+b, (e,c)] starts at db[e,c] ----
        ones1 = smallp.tile([1, B], BF16)
        nc.vector.memset(ones1[:], 1.0)
        psum_y = psy.tile([P, E, HC], F32)
        for h in range(2):
            nc.tensor.matmul(
                psum_y[h * 64:h * 64 + 64, :, :].rearrange("b e c -> b (e c)"),
                ones1[:],
                db_t[:, h * E * HC:(h + 1) * E * HC],
                start=True, stop=False, skip_group_check=True,
            )

        # Identity for the final PE transpose, built on the idle gpsimd.
        onesq = smallp.tile([P, P], F32)
        nc.gpsimd.memset(onesq[:], 1.0)
        idt_t = consts.tile([P, P], F32)
        nc.gpsimd.affine_select(
            out=idt_t[:], in_=onesq[:], pattern=[[-1, P]],
            compare_op=OP.is_equal, fill=0.0, base=0, channel_multiplier=1,
        )

        # ---- stage 1: y[64h+b, (e, hc)] += x . dense_w[e, oc_half, :] ----
        # k-outer so the PE consumes each w1 chunk as it lands.
        for k in range(KT):
            for h in range(2):
                nc.tensor.matmul(
                    psum_y[h * 64:h * 64 + 64, :, :].rearrange(
                        "b e c -> b (e c)"),
                    xt_t[:, k, :],
                    w1_t[:, k, h].rearrange("p e c -> p (e c)"),
                    start=False,
                    stop=(k == KT - 1),
                    skip_group_check=True,
                )

        prod_t = mixp.tile([P, E, HC], F32)
        nc.vector.tensor_tensor(
            out=prod_t[:], in0=psum_y[:], in1=gb_t[:], op=OP.mult,
        )
        # contiguous pairwise tree over e (strided reduce is ~2x slower)
        t1 = mixp.tile([P, 4, HC], F32)
        nc.vector.tensor_add(t1[:], prod_t[:, 0:4, :], prod_t[:, 4:8, :])
        t2 = mixp.tile([P, 2, HC], F32)
        nc.vector.tensor_add(t2[:], t1[:, 0:2, :], t1[:, 2:4, :])
        t3 = mixp.tile([P, HC], F32)
        nc.vector.tensor_add(t3[:], t2[:, 0, :], t2[:, 1, :])

        t_t = smallp.tile([P, HC], F32)
        nc.scalar.activation(t_t[:], t3[:], AF.Tanh)

        # ---- stage 2: pre[64h+b, l] = sum_hc t * sel_ow ----
        # NOTE: InstTensorTensorReduce faults TRN2; scalar_tensor_tensor with
        # accum_out (free-dim sum) is the reliable path.
        pre_t = smallp.tile([P, L], F32)
        dump = smallp.tile([P, HC], F32)
        for l in range(L):
            nc.vector.scalar_tensor_tensor(
                out=dump[:],
                in0=psum_ow[:, l, :],
                scalar=1.0,
                in1=t_t[:],
                op0=OP.mult,
                op1=OP.mult,
                accum_out=pre_t[:, l:l + 1],
            )
        # PE transpose [128,2] -> [2,128], accumulating onto the ob seed.
        nc.tensor.matmul(psum_o2[:], pre_t[:], idt_t[:],
                         is_transpose=True, start=False, stop=True,
                         skip_group_check=True)
        o2_t = smallp.tile([L, P], F32)
        nc.scalar.copy(o2_t[:], psum_o2[:])

        nc.sync.dma_start(out=out_d.ap(), in_=o2_t[:])

    nc.compile()
    return nc


def _prep_inputs(X, gates, dense_w, dense_b, out_w, out_b):
    """Host-side layout prep (slice/transpose/cast only) -> per-core maps."""
    import ml_dtypes

    BF = ml_dtypes.bfloat16
    X = np.asarray(X, dtype=np.float32)
    gates = np.asarray(gates, dtype=np.float32)
    dense_w = np.asarray(dense_w, dtype=np.float32)
    dense_b = np.asarray(dense_b, dtype=np.float32)
    out_w = np.asarray(out_w, dtype=np.float32)
    out_b = np.asarray(out_b, dtype=np.float32)

    xcls = X[:, 0, :]                                     # [B, H]
    # xt[i_lo, k, b] = x[b, k*128 + i_lo]
    xt = np.ascontiguousarray(
        xcls.T.reshape(KT, P, B).transpose(1, 0, 2)).astype(BF)
    gt = np.ascontiguousarray(gates.T)                    # [E, B]
    gtz = np.concatenate([gt, np.zeros_like(gt)], axis=1)  # [E, 128]
    gtdup = np.concatenate([gt, gt], axis=1)               # [E, 128]
    ebc = np.kron(np.eye(E, dtype=np.float32),
                  np.ones((1, HC), dtype=np.float32))      # [E, E*HC]

    in_maps = []
    for c in range(NCORES):
        sl = slice(c * OC, (c + 1) * OC)
        # w1[i_lo, k, h, e, hc] = dense_w[e, c*OC + h*64 + hc, k*128 + i_lo]
        w1 = np.ascontiguousarray(
            dense_w[:, sl, :]                   # [E, OC, H]
            .reshape(E, 2, HC, KT, P)           # [e, h, hc, k, i_lo]
            .transpose(4, 3, 1, 0, 2)           # [i_lo, k, h, e, hc]
        ).astype(BF)

        # dbp[0, (h, e, hc)] = dense_b[e, c*OC + h*64 + hc]
        dbp = np.ascontiguousarray(
            dense_b[:, sl].reshape(E, 2, HC).transpose(1, 0, 2).reshape(1, -1)
        ).astype(BF)

        # ow2[e, (h, l, hc)] = out_w[e, l, c*OC + h*64 + hc]
        ow2 = (out_w[:, :, sl].reshape(E, L, 2, HC)
               .transpose(0, 2, 1, 3).reshape(E, L * OC))
        ob = out_b if c == 0 else np.zeros_like(out_b)
        ep = np.ascontiguousarray(
            np.concatenate([gt, ow2, ob, gtz, gtdup, ebc], axis=1)
        )
        in_maps.append({
            "xt": xt,
            "w1": w1,
            "dbp": dbp,
            "ep": ep,
        })
    return in_maps


def _run(in_maps, trace=False, tmpdir=None):
    global _cached
    from concourse.bass_utils import run_bass_kernel_spmd

    if _cached is None:
        _cached = _build()
    res = run_bass_kernel_spmd(
        _cached, in_maps, list(range(NCORES)), trace=trace, tmpdir=tmpdir,
    )
    return res


def kernel(X, gates, dense_w, dense_b, out_w, out_b):
    in_maps = _prep_inputs(X, gates, dense_w, dense_b, out_w, out_b)
    res = _run(in_maps)
    acc = np.zeros((B, L), dtype=np.float64)
    for c in range(NCORES):
        part = res.results[c]["out"].astype(np.float64)   # [L, 128]
        acc += part.T.reshape(2, B, L).sum(axis=0)
    return acc.astype(np.float32)


# revision 5
# speedup vs baseline: 1.0694x; 1.0694x over previous
"""Trainium2 Bass kernel for nn_MoEsparseRoutingForClassification.

Reference computation (B=64, S=128, H=1024, E=8, L=2):
    x = X[:, 0, :]                                   # CLS token [B,H]
    y[b,o]   = sum_e g[b,e] * (x[b] . dense_w[e,o,:]) + (g @ dense_b)[b,o]
    t        = tanh(y)
    out[b,l] = sum_e g[b,e] * (t[b] . out_w[e,l,:])  + (g @ out_b)[b,l]

Distribution: the H output dim of the dense layer is sharded 8 ways
(OC=128 per core).  Core c computes y[:, c*OC:(c+1)*OC] (which needs the
full CLS token but only a slice dense_w[:, c_slice, :]), applies tanh,
and contracts its slice against out_w[:, :, c_slice] to produce a
partial [L,128] logit block.  The partials (incl. the out_b bias, fed
only to core 0) sum to the full output on the host.  No cross-core
collective is needed.

Everything that feeds the PE is bf16 (halves HBM traffic, single-pass
matmuls); PSUM accumulation and the DVE mix stay fp32.  rel-err budget
is 2e-2; bf16 rounding lands ~3e-3.

DMA: one ring (sync), ordered so each chain's completion unblocks work
just in time (DMA engines drain one descriptor chain before starting
the next; doorbell->data ~1.5us, dma-complete->sem-visible ~0.5us):
  cp (34 KB const pack: gates/out_w/biases/gate-broadcast consts, one
  sem for everything the small matmuls need) | xt (CLS) | w1 in 3
  chunks (3/3/2 k-tiles) that the PE chases.

PE program order: dense_b fold (K=1 matmul into psum_y, so the bias
needs no DVE add), sel_ow, out_b seed into the [2,128] output psum,
gate-broadcast table (K=8 matmul, replaces a 128-row gc DMA + 8 DVE
builds), then the 16 chunk-chasing stage-1 matmuls.  Post-mix chain:
mult + 3 tree adds + tanh + 2 accum-dots, then a PE transpose
accumulates the [128,2] result onto the out_b seed so the output DMA
is 2 rows x 512 B.

Everything arithmetic runs on device; the host only slices, transposes
(layout prep), and sums the partial outputs.
"""

import sys

import numpy as np

for _p in ("/opt/trn_rl_repo",):
    if _p not in sys.path:
        sys.path.insert(0, _p)

# If the environment sets BASS_TRACE but lacks antenv.axon_hooks (this agent
# image does), run_bass_kernel_spmd would crash on import; pre-seed a no-op
# module so tracing degrades gracefully instead.
try:  # pragma: no cover
    import antenv.axon_hooks  # noqa: F401
except Exception:  # pragma: no cover
    import types as _types

    _m = _types.ModuleType("antenv.axon_hooks")
    _m._hook = None
    _m.set_axon_ntff_profile_hook = lambda h: setattr(_m, "_hook", h)
    _m.get_axon_ntff_profile_hook = lambda: _m._hook
    sys.modules["antenv.axon_hooks"] = _m

B, S, H = 64, 128, 1024
E, L = 8, 2
NCORES = 8
OC = H // NCORES          # dense-output slice per core (128)
HC = OC // 2              # half-slice mapped to a PSUM partition half (64)
KT = H // 128             # contraction tiles
P = 128

W1_CHUNKS = ((0, 3), (3, 6), (6, 8))

# const-pack layout (bf16, [E, CPK]); row 0 additionally carries dense_b
OGT = 0                       # gates.T [E, B]
OOW = OGT + B                 # ow2 [E, 2*L*HC]
OOB = OOW + L * OC            # out_b [E, L] (zeros except core 0)
OGTZ = OOB + L                # gates.T | zeros [E, P]
OGTD = OGTZ + P               # gates.T | gates.T [E, P]
OEBC = OGTD + P               # kron(I_E, ones[HC]) [E, E*HC]
ODB = OEBC + E * HC           # dense_b row (row 0 only) [1, 2*E*HC]
CPK = ODB + 2 * E * HC

_cached = None


def _build():
    from contextlib import ExitStack

    import concourse.tile as tile
    from concourse import bacc, mybir

    F32 = mybir.dt.float32
    BF16 = mybir.dt.bfloat16
    AF = mybir.ActivationFunctionType
    OP = mybir.AluOpType

    nc = bacc.Bacc("TRN2", target_bir_lowering=False, debug=False,
                   num_devices=NCORES)

    xt_d = nc.dram_tensor("xt", [P, KT, B], BF16, kind="ExternalInput")
    w1_d = nc.dram_tensor("w1", [P, KT, 2, E, HC], BF16, kind="ExternalInput")
    cp_d = nc.dram_tensor("cp", [E, CPK], BF16, kind="ExternalInput")
    out_d = nc.dram_tensor("out", [L, P], F32, kind="ExternalOutput")

    with tile.TileContext(nc) as tc, ExitStack() as ctx:
        consts = ctx.enter_context(tc.tile_pool(name="consts", bufs=1))
        wpool = ctx.enter_context(tc.tile_pool(name="wpool", bufs=1))
        mixp = ctx.enter_context(tc.tile_pool(name="mixp", bufs=2))
        smallp = ctx.enter_context(tc.tile_pool(name="smallp", bufs=1))
        psy = ctx.enter_context(tc.tile_pool(name="psy", bufs=1, space="PSUM"))
        pss = ctx.enter_context(tc.tile_pool(name="pss", bufs=1, space="PSUM"))
        psg = ctx.enter_context(tc.tile_pool(name="psg", bufs=1, space="PSUM"))

        # Sync-ring order = DMA chain service order.
        cp_t = consts.tile([E, CPK], BF16)
        nc.sync.dma_start(out=cp_t, in_=cp_d.ap())
        xt_t = consts.tile([P, KT, B], BF16)
        nc.sync.dma_start(out=xt_t, in_=xt_d.ap())
        w1_t = wpool.tile([P, KT, 2, E, HC], BF16)
        for klo, khi in W1_CHUNKS:
            nc.sync.dma_start(
                out=w1_t[:, klo:khi],
                in_=w1_d.ap()[:, klo:khi],
            )

        gt_t = cp_t[:, OGT:OGT + B]
        ow_t = cp_t[:, OOW:OOW + L * OC].rearrange(
            "e (h l c) -> e h l c", h=2, l=L)
        ob_t = cp_t[:, OOB:OOB + L]
        gtz_t = cp_t[:, OGTZ:OGTZ + P]
        gtdup_t = cp_t[:, OGTD:OGTD + P]
        ebc_t = cp_t[:, OEBC:OEBC + E * HC]

        # ---- dense_b fold: psum_y[64h+b, (e,c)] starts at db[e,c] ----
        ones1 = smallp.tile([1, B], BF16)
        nc.vector.memset(ones1[:], 1.0)
        psum_y = psy.tile([P, E, HC], F32)
        for h in range(2):
            nc.tensor.matmul(
                psum_y[h * 64:h * 64 + 64, :, :].rearrange("b e c -> b (e c)"),
                ones1[:],
                cp_t[0:1, ODB + h * E * HC:ODB + (h + 1) * E * HC],
                start=True, stop=False, skip_group_check=True,
            )

        # ---- small matmuls (all bf16, one DMA sem) ----
        # sel_ow^h [64h+b, (l, hc)]
        psum_ow = pss.tile([P, L, HC], F32)
        for h in range(2):
            sl = slice(h * 64, h * 64 + 64)
            nc.tensor.matmul(
                psum_ow[sl, :, :].rearrange("b l c -> b (l c)"),
                gt_t, ow_t[:, h].rearrange("e l c -> e (l c)"),
                start=True, stop=True, skip_group_check=True,
            )
        # Output accumulator [l, p]: seed with sel_ob^T (only core 0
        # carries real ob); the stage-2 transpose accumulates on top.
        psum_o2 = pss.tile([L, P], F32)
        nc.tensor.matmul(psum_o2[:], ob_t, gtz_t,
                         start=True, stop=False, skip_group_check=True)
        # Gate-broadcast table gb[p, (e, hc)] = g[b, e] via one K=8 matmul.
        psum_gb = psg.tile([P, E, HC], F32)
        nc.tensor.matmul(psum_gb[:, :, :].rearrange("p e c -> p (e c)"),
                         gtdup_t, ebc_t, start=True, stop=True)
        gb_t = consts.tile([P, E, HC], F32)
        nc.vector.tensor_copy(gb_t[:], psum_gb[:])

        # Identity for the final PE transpose, built on the idle gpsimd.
        onesq = smallp.tile([P, P], F32)
        nc.gpsimd.memset(onesq[:], 1.0)
        idt_t = consts.tile([P, P], F32)
        nc.gpsimd.affine_select(
            out=idt_t[:], in_=onesq[:], pattern=[[-1, P]],
            compare_op=OP.is_equal, fill=0.0, base=0, channel_multiplier=1,
        )

        # ---- stage 1: y[64h+b, (e, hc)] += x . dense_w[e, oc_half, :] ----
        # k-outer so the PE consumes each w1 chunk as it lands.
        for k in range(KT):
            for h in range(2):
                nc.tensor.matmul(
                    psum_y[h * 64:h * 64 + 64, :, :].rearrange(
                        "b e c -> b (e c)"),
                    xt_t[:, k, :],
                    w1_t[:, k, h].rearrange("p e c -> p (e c)"),
                    start=False,
                    stop=(k == KT - 1),
                    skip_group_check=True,
                )

        prod_t = mixp.tile([P, E, HC], F32)
        nc.vector.tensor_tensor(
            out=prod_t[:], in0=psum_y[:], in1=gb_t[:], op=OP.mult,
        )
        # contiguous pairwise tree over e (strided reduce is ~2x slower)
        t1 = mixp.tile([P, 4, HC], F32)
        nc.vector.tensor_add(t1[:], prod_t[:, 0:4, :], prod_t[:, 4:8, :])
        t2 = mixp.tile([P, 2, HC], F32)
        nc.vector.tensor_add(t2[:], t1[:, 0:2, :], t1[:, 2:4, :])
        t3 = mixp.tile([P, HC], F32)
        nc.vector.tensor_add(t3[:], t2[:, 0, :], t2[:, 1, :])

        t_t = smallp.tile([P, HC], F32)
        nc.scalar.activation(t_t[:], t3[:], AF.Tanh)

        # ---- stage 2: pre[64h+b, l] = sum_hc t * sel_ow ----
        # NOTE: InstTensorTensorReduce faults TRN2; scalar_tensor_tensor with
        # accum_out (free-dim sum) is the reliable path.
        pre_t = smallp.tile([P, L], F32)
        dump = smallp.tile([P, HC], F32)
        for l in range(L):
            nc.vector.scalar_tensor_tensor(
                out=dump[:],
                in0=psum_ow[:, l, :],
                scalar=1.0,
                in1=t_t[:],
                op0=OP.mult,
                op1=OP.mult,
                accum_out=pre_t[:, l:l + 1],
            )
        # PE transpose [128,2] -> [2,128], accumulating onto the ob seed.
        nc.tensor.matmul(psum_o2[:], pre_t[:], idt_t[:],
                         is_transpose=True, start=False, stop=True,
                         skip_group_check=True)
        o2_t = smallp.tile([L, P], F32)
        nc.scalar.copy(o2_t[:], psum_o2[:])

        nc.sync.dma_start(out=out_d.ap(), in_=o2_t[:])

    nc.compile()
    return nc


def _prep_inputs(X, gates, dense_w, dense_b, out_w, out_b):
    """Host-side layout prep (slice/transpose/cast only) -> per-core maps."""
    import ml_dtypes

    BF = ml_dtypes.bfloat16
    X = np.asarray(X, dtype=np.float32)
    gates = np.asarray(gates, dtype=np.float32)
    dense_w = np.asarray(dense_w, dtype=np.float32)
    dense_b = np.asarray(dense_b, dtype=np.float32)
    out_w = np.asarray(out_w, dtype=np.float32)
    out_b = np.asarray(out_b, dtype=np.float32)

    xcls = X[:, 0, :]                                     # [B, H]
    # xt[i_lo, k, b] = x[b, k*128 + i_lo]
    xt = np.ascontiguousarray(
        xcls.T.reshape(KT, P, B).transpose(1, 0, 2)).astype(BF)
    gt = np.ascontiguousarray(gates.T)                    # [E, B]
    gtz = np.concatenate([gt, np.zeros_like(gt)], axis=1)  # [E, 128]
    gtdup = np.concatenate([gt, gt], axis=1)               # [E, 128]
    ebc = np.kron(np.eye(E, dtype=np.float32),
                  np.ones((1, HC), dtype=np.float32))      # [E, E*HC]

    in_maps = []
    for c in range(NCORES):
        sl = slice(c * OC, (c + 1) * OC)
        # w1[i_lo, k, h, e, hc] = dense_w[e, c*OC + h*64 + hc, k*128 + i_lo]
        w1 = np.ascontiguousarray(
            dense_w[:, sl, :]                   # [E, OC, H]
            .reshape(E, 2, HC, KT, P)           # [e, h, hc, k, i_lo]
            .transpose(4, 3, 1, 0, 2)           # [i_lo, k, h, e, hc]
        ).astype(BF)

        # db row: dense_b[e, c*OC + h*64 + hc] laid as (h, e, hc); row 0 only
        dbrow = np.zeros((E, 2 * E * HC), dtype=np.float32)
        dbrow[0] = (dense_b[:, sl].reshape(E, 2, HC)
                    .transpose(1, 0, 2).reshape(-1))

        # ow2[e, (h, l, hc)] = out_w[e, l, c*OC + h*64 + hc]
        ow2 = (out_w[:, :, sl].reshape(E, L, 2, HC)
               .transpose(0, 2, 1, 3).reshape(E, L * OC))
        ob = out_b if c == 0 else np.zeros_like(out_b)
        cp = np.ascontiguousarray(
            np.concatenate([gt, ow2, ob, gtz, gtdup, ebc, dbrow], axis=1)
        ).astype(BF)
        in_maps.append({
            "xt": xt,
            "w1": w1,
            "cp": cp,
        })
    return in_maps


def _run(in_maps, trace=False, tmpdir=None):
    global _cached
    from concourse.bass_utils import run_bass_kernel_spmd

    if _cached is None:
        _cached = _build()
    res = run_bass_kernel_spmd(
        _cached, in_maps, list(range(NCORES)), trace=trace, tmpdir=tmpdir,
    )
    return res


def kernel(X, gates, dense_w, dense_b, out_w, out_b):
    in_maps = _prep_inputs(X, gates, dense_w, dense_b, out_w, out_b)
    res = _run(in_maps)
    acc = np.zeros((B, L), dtype=np.float64)
    for c in range(NCORES):
        part = res.results[c]["out"].astype(np.float64)   # [L, 128]
        acc += part.T.reshape(2, B, L).sum(axis=0)
    return acc.astype(np.float32)


# revision 7
# speedup vs baseline: 1.1262x; 1.0531x over previous
"""Trainium2 Bass kernel for nn_MoEsparseRoutingForClassification.

Reference computation (B=64, S=128, H=1024, E=8, L=2):
    x = X[:, 0, :]                                   # CLS token [B,H]
    y[b,o]   = sum_e g[b,e] * (x[b] . dense_w[e,o,:]) + (g @ dense_b)[b,o]
    t        = tanh(y)
    out[b,l] = sum_e g[b,e] * (t[b] . out_w[e,l,:])  + (g @ out_b)[b,l]

Distribution: the H output dim of the dense layer is sharded 8 ways
(OC=128 per core).  Core c computes y[:, c*OC:(c+1)*OC] (which needs the
full CLS token but only a slice dense_w[:, c_slice, :]), applies tanh,
and contracts its slice against out_w[:, :, c_slice] to produce a
partial [L,128] logit block.  The partials (incl. the out_b bias, fed
only to core 0) sum to the full output on the host.  No cross-core
collective is needed.

Everything that feeds the PE is bf16 (halves HBM traffic, single-pass
matmuls); PSUM accumulation and the DVE mix stay fp32.  rel-err budget
is 2e-2; bf16 rounding lands ~3e-3.

DMA: one ring (sync), ordered so each chain's completion unblocks work
just in time (DMA engines drain one descriptor chain before starting
the next; doorbell->data ~1.5us, dma-complete->sem-visible ~0.5us):
  cp (34 KB const pack: gates/out_w/biases/gate-broadcast consts, one
  sem for everything the small matmuls need) | xt (CLS) | w1 in 3
  chunks (3/3/2 k-tiles) that the PE chases.

PE program order: dense_b fold (K=1 matmul into psum_y, so the bias
needs no DVE add), sel_ow, out_b seed into the [2,128] output psum,
gate-broadcast table (K=8 matmul, replaces a 128-row gc DMA + 8 DVE
builds), then the 16 chunk-chasing stage-1 matmuls.  Post-mix chain:
mult + 3 tree adds + tanh + 2 accum-dots, then a PE transpose
accumulates the [128,2] result onto the out_b seed so the output DMA
is 2 rows x 512 B.

Everything arithmetic runs on device; the host only slices, transposes
(layout prep), and sums the partial outputs.
"""

import sys

import numpy as np

for _p in ("/opt/trn_rl_repo",):
    if _p not in sys.path:
        sys.path.insert(0, _p)

# If the environment sets BASS_TRACE but lacks antenv.axon_hooks (this agent
# image does), run_bass_kernel_spmd would crash on import; pre-seed a no-op
# module so tracing degrades gracefully instead.
try:  # pragma: no cover
    import antenv.axon_hooks  # noqa: F401
except Exception:  # pragma: no cover
    import types as _types

    _m = _types.ModuleType("antenv.axon_hooks")
    _m._hook = None
    _m.set_axon_ntff_profile_hook = lambda h: setattr(_m, "_hook", h)
    _m.get_axon_ntff_profile_hook = lambda: _m._hook
    sys.modules["antenv.axon_hooks"] = _m

B, S, H = 64, 128, 1024
E, L = 8, 2
NCORES = 8
OC = H // NCORES          # dense-output slice per core (128)
HC = OC // 2              # half-slice mapped to a PSUM partition half (64)
KT = H // 128             # contraction tiles
P = 128

W1_CHUNKS = ((0, 4), (4, 7), (7, 8))

# const-pack layout (bf16, [E, CPK]); row 0 additionally carries dense_b
OGT = 0                       # gates.T [E, B]
OOW = OGT + B                 # ow2 [E, 2*L*HC]
OOB = OOW + L * OC            # out_b [E, L] (zeros except core 0)
OGTZ = OOB + L                # gates.T | zeros [E, P]
OGTD = OGTZ + P               # gates.T | gates.T [E, P]
OEBC = OGTD + P               # kron(I_E, ones[HC]) [E, E*HC]
ODB = OEBC + E * HC           # dense_b row (row 0 only) [1, 2*E*HC]
CPK = ODB + 2 * E * HC

_cached = None


def _build():
    from contextlib import ExitStack

    import concourse.tile as tile
    from concourse import bacc, mybir

    F32 = mybir.dt.float32
    BF16 = mybir.dt.bfloat16
    AF = mybir.ActivationFunctionType
    OP = mybir.AluOpType

    nc = bacc.Bacc("TRN2", target_bir_lowering=False, debug=False,
                   num_devices=NCORES)

    xt_d = nc.dram_tensor("xt", [P, KT, B], BF16, kind="ExternalInput")
    w1_d = nc.dram_tensor("w1", [P, KT, 2, E, HC], BF16, kind="ExternalInput")
    cp_d = nc.dram_tensor("cp", [E, CPK], BF16, kind="ExternalInput")
    out_d = nc.dram_tensor("out", [L, P], F32, kind="ExternalOutput")

    with tile.TileContext(nc) as tc, ExitStack() as ctx:
        consts = ctx.enter_context(tc.tile_pool(name="consts", bufs=1))
        wpool = ctx.enter_context(tc.tile_pool(name="wpool", bufs=1))
        mixp = ctx.enter_context(tc.tile_pool(name="mixp", bufs=2))
        smallp = ctx.enter_context(tc.tile_pool(name="smallp", bufs=1))
        psy = ctx.enter_context(tc.tile_pool(name="psy", bufs=1, space="PSUM"))
        pss = ctx.enter_context(tc.tile_pool(name="pss", bufs=1, space="PSUM"))
        psg = ctx.enter_context(tc.tile_pool(name="psg", bufs=1, space="PSUM"))

        # Sync-ring order = DMA chain service order.
        cp_t = consts.tile([E, CPK], BF16)
        nc.sync.dma_start(out=cp_t, in_=cp_d.ap())
        xt_t = consts.tile([P, KT, B], BF16)
        nc.sync.dma_start(out=xt_t, in_=xt_d.ap())
        w1_t = wpool.tile([P, KT, 2, E, HC], BF16)
        for klo, khi in W1_CHUNKS:
            nc.sync.dma_start(
                out=w1_t[:, klo:khi],
                in_=w1_d.ap()[:, klo:khi],
            )

        gt_t = cp_t[:, OGT:OGT + B]
        ow_t = cp_t[:, OOW:OOW + L * OC].rearrange(
            "e (h l c) -> e h l c", h=2, l=L)
        ob_t = cp_t[:, OOB:OOB + L]
        gtz_t = cp_t[:, OGTZ:OGTZ + P]
        gtdup_t = cp_t[:, OGTD:OGTD + P]
        ebc_t = cp_t[:, OEBC:OEBC + E * HC]

        # ---- dense_b fold: psum_y[64h+b, (e,c)] starts at db[e,c] ----
        ones1 = smallp.tile([1, B], BF16)
        nc.vector.memset(ones1[:], 1.0)
        psum_y = psy.tile([P, E, HC], F32)
        for h in range(2):
            nc.tensor.matmul(
                psum_y[h * 64:h * 64 + 64, :, :].rearrange("b e c -> b (e c)"),
                ones1[:],
                cp_t[0:1, ODB + h * E * HC:ODB + (h + 1) * E * HC],
                start=True, stop=False, skip_group_check=True,
            )

        # ---- small matmuls (all bf16, one DMA sem) ----
        # sel_ow^h [64h+b, (l, hc)]
        psum_ow = pss.tile([P, L, HC], F32)
        for h in range(2):
            sl = slice(h * 64, h * 64 + 64)
            nc.tensor.matmul(
                psum_ow[sl, :, :].rearrange("b l c -> b (l c)"),
                gt_t, ow_t[:, h].rearrange("e l c -> e (l c)"),
                start=True, stop=True, skip_group_check=True,
            )
        # Output accumulator [l, p]: seed with sel_ob^T (only core 0
        # carries real ob); the stage-2 transpose accumulates on top.
        psum_o2 = pss.tile([L, P], F32)
        nc.tensor.matmul(psum_o2[:], ob_t, gtz_t,
                         start=True, stop=False, skip_group_check=True)
        # Gate-broadcast table gb[p, (e, hc)] = g[b, e] via one K=8 matmul.
        psum_gb = psg.tile([P, E, HC], F32)
        nc.tensor.matmul(psum_gb[:, :, :].rearrange("p e c -> p (e c)"),
                         gtdup_t, ebc_t, start=True, stop=True)
        gb_t = consts.tile([P, E, HC], F32)
        nc.vector.tensor_copy(gb_t[:], psum_gb[:])

        # Identity for the final PE transpose, built on the idle gpsimd.
        onesq = smallp.tile([P, P], F32)
        nc.gpsimd.memset(onesq[:], 1.0)
        idt_t = consts.tile([P, P], F32)
        nc.gpsimd.affine_select(
            out=idt_t[:], in_=onesq[:], pattern=[[-1, P]],
            compare_op=OP.is_equal, fill=0.0, base=0, channel_multiplier=1,
        )

        # ---- stage 1: y[64h+b, (e, hc)] += x . dense_w[e, oc_half, :] ----
        # k-outer so the PE consumes each w1 chunk as it lands.
        for k in range(KT):
            for h in range(2):
                nc.tensor.matmul(
                    psum_y[h * 64:h * 64 + 64, :, :].rearrange(
                        "b e c -> b (e c)"),
                    xt_t[:, k, :],
                    w1_t[:, k, h].rearrange("p e c -> p (e c)"),
                    start=False,
                    stop=(k == KT - 1),
                    skip_group_check=True,
                )

        prod_t = mixp.tile([P, E, HC], F32)
        nc.vector.tensor_tensor(
            out=prod_t[:], in0=psum_y[:], in1=gb_t[:], op=OP.mult,
        )
        # contiguous pairwise tree over e (strided reduce is ~2x slower)
        t1 = mixp.tile([P, 4, HC], F32)
        nc.vector.tensor_add(t1[:], prod_t[:, 0:4, :], prod_t[:, 4:8, :])
        t2 = mixp.tile([P, 2, HC], F32)
        nc.vector.tensor_add(t2[:], t1[:, 0:2, :], t1[:, 2:4, :])
        t3 = mixp.tile([P, HC], F32)
        nc.vector.tensor_add(t3[:], t2[:, 0, :], t2[:, 1, :])

        t_t = smallp.tile([P, HC], F32)
        nc.scalar.activation(t_t[:], t3[:], AF.Tanh)

        # ---- stage 2: pre[64h+b, l] = sum_hc t * sel_ow ----
        # NOTE: InstTensorTensorReduce faults TRN2; scalar_tensor_tensor with
        # accum_out (free-dim sum) is the reliable path.
        pre_t = smallp.tile([P, L], F32)
        dump = smallp.tile([P, HC], F32)
        for l in range(L):
            nc.vector.scalar_tensor_tensor(
                out=dump[:],
                in0=psum_ow[:, l, :],
                scalar=1.0,
                in1=t_t[:],
                op0=OP.mult,
                op1=OP.mult,
                accum_out=pre_t[:, l:l + 1],
            )
        # PE transpose [128,2] -> [2,128], accumulating onto the ob seed.
        nc.tensor.matmul(psum_o2[:], pre_t[:], idt_t[:],
                         is_transpose=True, start=False, stop=True,
                         skip_group_check=True)
        o2_t = smallp.tile([L, P], F32)
        nc.vector.tensor_copy(o2_t[:], psum_o2[:])

        nc.sync.dma_start(out=out_d.ap(), in_=o2_t[:])

    nc.compile()
    return nc


def _prep_inputs(X, gates, dense_w, dense_b, out_w, out_b):
    """Host-side layout prep (slice/transpose/cast only) -> per-core maps."""
    import ml_dtypes

    BF = ml_dtypes.bfloat16
    X = np.asarray(X, dtype=np.float32)
    gates = np.asarray(gates, dtype=np.float32)
    dense_w = np.asarray(dense_w, dtype=np.float32)
    dense_b = np.asarray(dense_b, dtype=np.float32)
    out_w = np.asarray(out_w, dtype=np.float32)
    out_b = np.asarray(out_b, dtype=np.float32)

    xcls = X[:, 0, :]                                     # [B, H]
    # xt[i_lo, k, b] = x[b, k*128 + i_lo]
    xt = np.ascontiguousarray(
        xcls.T.reshape(KT, P, B).transpose(1, 0, 2)).astype(BF)
    gt = np.ascontiguousarray(gates.T)                    # [E, B]
    gtz = np.concatenate([gt, np.zeros_like(gt)], axis=1)  # [E, 128]
    gtdup = np.concatenate([gt, gt], axis=1)               # [E, 128]
    ebc = np.kron(np.eye(E, dtype=np.float32),
                  np.ones((1, HC), dtype=np.float32))      # [E, E*HC]

    in_maps = []
    for c in range(NCORES):
        sl = slice(c * OC, (c + 1) * OC)
        # w1[i_lo, k, h, e, hc] = dense_w[e, c*OC + h*64 + hc, k*128 + i_lo]
        w1 = np.ascontiguousarray(
            dense_w[:, sl, :]                   # [E, OC, H]
            .reshape(E, 2, HC, KT, P)           # [e, h, hc, k, i_lo]
            .transpose(4, 3, 1, 0, 2)           # [i_lo, k, h, e, hc]
        ).astype(BF)

        # db row: dense_b[e, c*OC + h*64 + hc] laid as (h, e, hc); row 0 only
        dbrow = np.zeros((E, 2 * E * HC), dtype=np.float32)
        dbrow[0] = (dense_b[:, sl].reshape(E, 2, HC)
                    .transpose(1, 0, 2).reshape(-1))

        # ow2[e, (h, l, hc)] = out_w[e, l, c*OC + h*64 + hc]
        ow2 = (out_w[:, :, sl].reshape(E, L, 2, HC)
               .transpose(0, 2, 1, 3).reshape(E, L * OC))
        ob = out_b if c == 0 else np.zeros_like(out_b)
        cp = np.ascontiguousarray(
            np.concatenate([gt, ow2, ob, gtz, gtdup, ebc, dbrow], axis=1)
        ).astype(BF)
        in_maps.append({
            "xt": xt,
            "w1": w1,
            "cp": cp,
        })
    return in_maps


def _run(in_maps, trace=False, tmpdir=None):
    global _cached
    from concourse.bass_utils import run_bass_kernel_spmd

    if _cached is None:
        _cached = _build()
    res = run_bass_kernel_spmd(
        _cached, in_maps, list(range(NCORES)), trace=trace, tmpdir=tmpdir,
    )
    return res


def kernel(X, gates, dense_w, dense_b, out_w, out_b):
    in_maps = _prep_inputs(X, gates, dense_w, dense_b, out_w, out_b)
    res = _run(in_maps)
    acc = np.zeros((B, L), dtype=np.float64)
    for c in range(NCORES):
        part = res.results[c]["out"].astype(np.float64)   # [L, 128]
        acc += part.T.reshape(2, B, L).sum(axis=0)
    return acc.astype(np.float32)


# revision 13
# speedup vs baseline: 1.1313x; 1.0045x over previous
"""Trainium2 Bass kernel for nn_MoEsparseRoutingForClassification.

Reference computation (B=64, S=128, H=1024, E=8, L=2):
    x = X[:, 0, :]                                   # CLS token [B,H]
    y[b,o]   = sum_e g[b,e] * (x[b] . dense_w[e,o,:]) + (g @ dense_b)[b,o]
    t        = tanh(y)
    out[b,l] = sum_e g[b,e] * (t[b] . out_w[e,l,:])  + (g @ out_b)[b,l]

Distribution: the H output dim of the dense layer is sharded 8 ways
(OC=128 per core).  Core c computes y[:, c*OC:(c+1)*OC] (which needs the
full CLS token but only a slice dense_w[:, c_slice, :]), applies tanh,
and contracts its slice against out_w[:, :, c_slice] to produce a
partial [L,128] logit block.  The partials (incl. the out_b bias, fed
only to core 0) sum to the full output on the host.  No cross-core
collective is needed.

Everything that feeds the PE is bf16 (halves HBM traffic, single-pass
matmuls); PSUM accumulation and the DVE mix stay fp32.  rel-err budget
is 2e-2; bf16 rounding lands ~3e-3.

DMA: one ring (sync), ordered so each chain's completion unblocks work
just in time (DMA engines drain one descriptor chain before starting
the next; doorbell->data ~1.5us, dma-complete->sem-visible ~0.5us):
  cp (34 KB const pack: gates/out_w/biases/gate-broadcast consts, one
  sem for everything the small matmuls need) | xt (CLS) | w1 in 3
  chunks (3/3/2 k-tiles) that the PE chases.

PE program order: dense_b fold (K=1 matmul into psum_y, so the bias
needs no DVE add), sel_ow, out_b seed into the [2,128] output psum,
gate-broadcast table (K=8 matmul, replaces a 128-row gc DMA + 8 DVE
builds), then the 16 chunk-chasing stage-1 matmuls.  Post-mix chain:
mult + 3 tree adds + tanh + 2 accum-dots, then a PE transpose
accumulates the [128,2] result onto the out_b seed so the output DMA
is 2 rows x 512 B.

Everything arithmetic runs on device; the host only slices, transposes
(layout prep), and sums the partial outputs.
"""

import sys

import numpy as np

for _p in ("/opt/trn_rl_repo",):
    if _p not in sys.path:
        sys.path.insert(0, _p)

# If the environment sets BASS_TRACE but lacks antenv.axon_hooks (this agent
# image does), run_bass_kernel_spmd would crash on import; pre-seed a no-op
# module so tracing degrades gracefully instead.
try:  # pragma: no cover
    import antenv.axon_hooks  # noqa: F401
except Exception:  # pragma: no cover
    import types as _types

    _m = _types.ModuleType("antenv.axon_hooks")
    _m._hook = None
    _m.set_axon_ntff_profile_hook = lambda h: setattr(_m, "_hook", h)
    _m.get_axon_ntff_profile_hook = lambda: _m._hook
    sys.modules["antenv.axon_hooks"] = _m

B, S, H = 64, 128, 1024
E, L = 8, 2
NCORES = 8
OC = H // NCORES          # dense-output slice per core (128)
HC = OC // 2              # half-slice mapped to a PSUM partition half (64)
KT = H // 128             # contraction tiles
P = 128

# Combined xt|w1 stream chunk boundaries, in bf16 elements per partition:
# xt occupies [0, 512); w1 k-tile k occupies [512 + 1024*k, 512 + 1024*(k+1)).
WX = KT * B + KT * 2 * E * (OC // 2)   # 512 + 8192
WX_CHUNKS = ((0, 4608), (4608, 7680), (7680, 8704))   # xt+k0-3 | k4-6 | k7

# const-pack layout (bf16, [E, CPK]); row 0 additionally carries dense_b
OGT = 0                       # gates.T [E, B]
OOW = OGT + B                 # ow2 [E, 2*L*HC]
OOB = OOW + L * OC            # out_b [E, L] (zeros except core 0)
OGTZ = OOB + L                # gates.T | zeros [E, P]
OGTD = OGTZ + P               # gates.T | gates.T [E, P]
OEBC = OGTD + P               # kron(I_E, ones[HC]) [E, E*HC]
ODB = OEBC + E * HC           # dense_b row (row 0 only) [1, 2*E*HC]
CPK = ODB + 2 * E * HC

_cached = None


def _build():
    from contextlib import ExitStack

    import concourse.tile as tile
    from concourse import bacc, mybir

    F32 = mybir.dt.float32
    BF16 = mybir.dt.bfloat16
    AF = mybir.ActivationFunctionType
    OP = mybir.AluOpType

    nc = bacc.Bacc("TRN2", target_bir_lowering=False, debug=False,
                   num_devices=NCORES)

    wx_d = nc.dram_tensor("wx", [P, WX], BF16, kind="ExternalInput")
    cp_d = nc.dram_tensor("cp", [E, CPK], BF16, kind="ExternalInput")
    out_d = nc.dram_tensor("out", [L, P], F32, kind="ExternalOutput")

    with tile.TileContext(nc) as tc, ExitStack() as ctx:
        consts = ctx.enter_context(tc.tile_pool(name="consts", bufs=1))
        wpool = ctx.enter_context(tc.tile_pool(name="wpool", bufs=1))
        mixp = ctx.enter_context(tc.tile_pool(name="mixp", bufs=2))
        smallp = ctx.enter_context(tc.tile_pool(name="smallp", bufs=1))
        psy = ctx.enter_context(tc.tile_pool(name="psy", bufs=1, space="PSUM"))
        pss = ctx.enter_context(tc.tile_pool(name="pss", bufs=1, space="PSUM"))
        psg = ctx.enter_context(tc.tile_pool(name="psg", bufs=1, space="PSUM"))

        # Sync-ring order = DMA chain service order.  xt rides at the head
        # of the combined wx stream so it shares chunk 0's chain/sem.
        cp_t = consts.tile([E, CPK], BF16)
        nc.sync.dma_start(out=cp_t, in_=cp_d.ap())
        wx_t = wpool.tile([P, WX], BF16)
        for lo, hi in WX_CHUNKS:
            nc.sync.dma_start(
                out=wx_t[:, lo:hi],
                in_=wx_d.ap()[:, lo:hi],
            )
        xt_t = wx_t[:, 0:KT * B].rearrange("p (k b) -> p k b", k=KT)
        w1_t = wx_t[:, KT * B:].rearrange(
            "p (k h e c) -> p k h e c", k=KT, h=2, e=E)

        gt_t = cp_t[:, OGT:OGT + B]
        ow_t = cp_t[:, OOW:OOW + L * OC].rearrange(
            "e (h l c) -> e h l c", h=2, l=L)
        ob_t = cp_t[:, OOB:OOB + L]
        gtz_t = cp_t[:, OGTZ:OGTZ + P]
        gtdup_t = cp_t[:, OGTD:OGTD + P]
        ebc_t = cp_t[:, OEBC:OEBC + E * HC]

        # ---- dense_b fold: psum_y[64h+b, (e,c)] starts at db[e,c] ----
        ones1 = smallp.tile([1, B], BF16)
        nc.vector.memset(ones1[:], 1.0)
        psum_y = psy.tile([P, E, HC], F32)
        for h in range(2):
            nc.tensor.matmul(
                psum_y[h * 64:h * 64 + 64, :, :].rearrange("b e c -> b (e c)"),
                ones1[:],
                cp_t[0:1, ODB + h * E * HC:ODB + (h + 1) * E * HC],
                start=True, stop=False, skip_group_check=True,
            )

        # ---- small matmuls (all bf16, one DMA sem) ----
        # sel_ow^h [64h+b, (l, hc)]
        psum_ow = pss.tile([P, L, HC], F32)
        for h in range(2):
            sl = slice(h * 64, h * 64 + 64)
            nc.tensor.matmul(
                psum_ow[sl, :, :].rearrange("b l c -> b (l c)"),
                gt_t, ow_t[:, h].rearrange("e l c -> e (l c)"),
                start=True, stop=True, skip_group_check=True,
            )
        # Output accumulator [l, p]: seed with sel_ob^T (only core 0
        # carries real ob); the stage-2 transpose accumulates on top.
        psum_o2 = pss.tile([L, P], F32)
        nc.tensor.matmul(psum_o2[:], ob_t, gtz_t,
                         start=True, stop=False, skip_group_check=True)
        # Gate-broadcast table gb[p, (e, hc)] = g[b, e] via one K=8 matmul.
        psum_gb = psg.tile([P, E, HC], F32)
        nc.tensor.matmul(psum_gb[:, :, :].rearrange("p e c -> p (e c)"),
                         gtdup_t, ebc_t, start=True, stop=True)
        gb_t = consts.tile([P, E, HC], F32)
        nc.vector.tensor_copy(gb_t[:], psum_gb[:])

        # Identity for the final PE transpose, built on the idle gpsimd.
        onesq = smallp.tile([P, P], F32)
        nc.gpsimd.memset(onesq[:], 1.0)
        idt_t = consts.tile([P, P], F32)
        nc.gpsimd.affine_select(
            out=idt_t[:], in_=onesq[:], pattern=[[-1, P]],
            compare_op=OP.is_equal, fill=0.0, base=0, channel_multiplier=1,
        )

        # ---- stage 1: y[64h+b, (e, hc)] += x . dense_w[e, oc_half, :] ----
        # k-outer so the PE consumes each w1 chunk as it lands.
        for k in range(KT):
            for h in range(2):
                nc.tensor.matmul(
                    psum_y[h * 64:h * 64 + 64, :, :].rearrange(
                        "b e c -> b (e c)"),
                    xt_t[:, k, :],
                    w1_t[:, k, h].rearrange("p e c -> p (e c)"),
                    start=False,
                    stop=(k == KT - 1),
                    skip_group_check=True,
                )

        prod_t = mixp.tile([P, E, HC], F32)
        nc.vector.tensor_tensor(
            out=prod_t[:], in0=psum_y[:], in1=gb_t[:], op=OP.mult,
        )
        # contiguous pairwise tree over e (strided reduce is ~2x slower)
        t1 = mixp.tile([P, 4, HC], F32)
        nc.vector.tensor_add(t1[:], prod_t[:, 0:4, :], prod_t[:, 4:8, :])
        t2 = mixp.tile([P, 2, HC], F32)
        nc.vector.tensor_add(t2[:], t1[:, 0:2, :], t1[:, 2:4, :])
        t3 = mixp.tile([P, HC], F32)
        nc.vector.tensor_add(t3[:], t2[:, 0, :], t2[:, 1, :])

        t_t = smallp.tile([P, HC], F32)
        nc.scalar.activation(t_t[:], t3[:], AF.Tanh)

        # ---- stage 2: pre[64h+b, l] = sum_hc t * sel_ow ----
        # NOTE: InstTensorTensorReduce faults TRN2; scalar_tensor_tensor with
        # accum_out (free-dim sum) is the reliable path.
        pre_t = smallp.tile([P, L], F32)
        dump = smallp.tile([P, HC], F32)
        for l in range(L):
            nc.vector.scalar_tensor_tensor(
                out=dump[:],
                in0=psum_ow[:, l, :],
                scalar=1.0,
                in1=t_t[:],
                op0=OP.mult,
                op1=OP.mult,
                accum_out=pre_t[:, l:l + 1],
            )
        # PE transpose [128,2] -> [2,128], accumulating onto the ob seed.
        nc.tensor.matmul(psum_o2[:], pre_t[:], idt_t[:],
                         is_transpose=True, start=False, stop=True,
                         skip_group_check=True)
        o2_t = smallp.tile([L, P], F32)
        nc.vector.tensor_copy(o2_t[:], psum_o2[:])

        nc.sync.dma_start(out=out_d.ap(), in_=o2_t[:])

    nc.compile()
    return nc


def _prep_inputs(X, gates, dense_w, dense_b, out_w, out_b):
    """Host-side layout prep (slice/transpose/cast only) -> per-core maps."""
    import ml_dtypes

    BF = ml_dtypes.bfloat16
    X = np.asarray(X, dtype=np.float32)
    gates = np.asarray(gates, dtype=np.float32)
    dense_w = np.asarray(dense_w, dtype=np.float32)
    dense_b = np.asarray(dense_b, dtype=np.float32)
    out_w = np.asarray(out_w, dtype=np.float32)
    out_b = np.asarray(out_b, dtype=np.float32)

    xcls = X[:, 0, :]                                     # [B, H]
    # xt[i_lo, k, b] = x[b, k*128 + i_lo]
    xt = np.ascontiguousarray(
        xcls.T.reshape(KT, P, B).transpose(1, 0, 2)).astype(BF)
    xt_flat = xt.reshape(P, KT * B)
    gt = np.ascontiguousarray(gates.T)                    # [E, B]
    gtz = np.concatenate([gt, np.zeros_like(gt)], axis=1)  # [E, 128]
    gtdup = np.concatenate([gt, gt], axis=1)               # [E, 128]
    ebc = np.kron(np.eye(E, dtype=np.float32),
                  np.ones((1, HC), dtype=np.float32))      # [E, E*HC]

    in_maps = []
    for c in range(NCORES):
        sl = slice(c * OC, (c + 1) * OC)
        # w1[i_lo, k, h, e, hc] = dense_w[e, c*OC + h*64 + hc, k*128 + i_lo]
        w1 = np.ascontiguousarray(
            dense_w[:, sl, :]                   # [E, OC, H]
            .reshape(E, 2, HC, KT, P)           # [e, h, hc, k, i_lo]
            .transpose(4, 3, 1, 0, 2)           # [i_lo, k, h, e, hc]
        ).astype(BF)
        wx = np.ascontiguousarray(
            np.concatenate([xt_flat, w1.reshape(P, KT * 2 * E * HC)], axis=1))

        # db row: dense_b[e, c*OC + h*64 + hc] laid as (h, e, hc); row 0 only
        dbrow = np.zeros((E, 2 * E * HC), dtype=np.float32)
        dbrow[0] = (dense_b[:, sl].reshape(E, 2, HC)
                    .transpose(1, 0, 2).reshape(-1))

        # ow2[e, (h, l, hc)] = out_w[e, l, c*OC + h*64 + hc]
        ow2 = (out_w[:, :, sl].reshape(E, L, 2, HC)
               .transpose(0, 2, 1, 3).reshape(E, L * OC))
        ob = out_b if c == 0 else np.zeros_like(out_b)
        cp = np.ascontiguousarray(
            np.concatenate([gt, ow2, ob, gtz, gtdup, ebc, dbrow], axis=1)
        ).astype(BF)
        in_maps.append({
            "wx": wx,
            "cp": cp,
        })
    return in_maps


def _run(in_maps, trace=False, tmpdir=None):
    global _cached
    from concourse.bass_utils import run_bass_kernel_spmd

    if _cached is None:
        _cached = _build()
    res = run_bass_kernel_spmd(
        _cached, in_maps, list(range(NCORES)), trace=trace, tmpdir=tmpdir,
    )
    return res


def kernel(X, gates, dense_w, dense_b, out_w, out_b):
    in_maps = _prep_inputs(X, gates, dense_w, dense_b, out_w, out_b)
    res = _run(in_maps)
    acc = np.zeros((B, L), dtype=np.float64)
    for c in range(NCORES):
        part = res.results[c]["out"].astype(np.float64)   # [L, 128]
        acc += part.T.reshape(2, B, L).sum(axis=0)
    return acc.astype(np.float32)


# revision 14
# speedup vs baseline: 1.1320x; 1.0006x over previous
"""Trainium2 Bass kernel for nn_MoEsparseRoutingForClassification.

Reference computation (B=64, S=128, H=1024, E=8, L=2):
    x = X[:, 0, :]                                   # CLS token [B,H]
    y[b,o]   = sum_e g[b,e] * (x[b] . dense_w[e,o,:]) + (g @ dense_b)[b,o]
    t        = tanh(y)
    out[b,l] = sum_e g[b,e] * (t[b] . out_w[e,l,:])  + (g @ out_b)[b,l]

Distribution: the H output dim of the dense layer is sharded 8 ways
(OC=128 per core).  Core c computes y[:, c*OC:(c+1)*OC] (which needs the
full CLS token but only a slice dense_w[:, c_slice, :]), applies tanh,
and contracts its slice against out_w[:, :, c_slice] to produce a
partial [L,128] logit block.  The partials (incl. the out_b bias, fed
only to core 0) sum to the full output on the host.  No cross-core
collective is needed.

Everything that feeds the PE is bf16 (halves HBM traffic, single-pass
matmuls); PSUM accumulation and the DVE mix stay fp32.  rel-err budget
is 2e-2; bf16 rounding lands ~3e-3.

DMA: one ring (sync), ordered so each chain's completion unblocks work
just in time (DMA engines drain one descriptor chain before starting
the next; doorbell->data ~1.5us, dma-complete->sem-visible ~0.5us):
  cp (34 KB const pack: gates/out_w/biases/gate-broadcast consts, one
  sem for everything the small matmuls need) | xt (CLS) | w1 in 3
  chunks (3/3/2 k-tiles) that the PE chases.

PE program order: dense_b fold (K=1 matmul into psum_y, so the bias
needs no DVE add), sel_ow, out_b seed into the [2,128] output psum,
gate-broadcast table (K=8 matmul, replaces a 128-row gc DMA + 8 DVE
builds), then the 16 chunk-chasing stage-1 matmuls.  Post-mix chain:
mult + 3 tree adds + tanh + 2 accum-dots, then a PE transpose
accumulates the [128,2] result onto the out_b seed so the output DMA
is 2 rows x 512 B.

Everything arithmetic runs on device; the host only slices, transposes
(layout prep), and sums the partial outputs.
"""

import sys

import numpy as np

for _p in ("/opt/trn_rl_repo",):
    if _p not in sys.path:
        sys.path.insert(0, _p)

# If the environment sets BASS_TRACE but lacks antenv.axon_hooks (this agent
# image does), run_bass_kernel_spmd would crash on import; pre-seed a no-op
# module so tracing degrades gracefully instead.
try:  # pragma: no cover
    import antenv.axon_hooks  # noqa: F401
except Exception:  # pragma: no cover
    import types as _types

    _m = _types.ModuleType("antenv.axon_hooks")
    _m._hook = None
    _m.set_axon_ntff_profile_hook = lambda h: setattr(_m, "_hook", h)
    _m.get_axon_ntff_profile_hook = lambda: _m._hook
    sys.modules["antenv.axon_hooks"] = _m

B, S, H = 64, 128, 1024
E, L = 8, 2
NCORES = 8
OC = H // NCORES          # dense-output slice per core (128)
HC = OC // 2              # half-slice mapped to a PSUM partition half (64)
KT = H // 128             # contraction tiles
P = 128

# Combined xt|w1 stream chunk boundaries, in bf16 elements per partition:
# xt occupies [0, 512); w1 k-tile k occupies [512 + 1024*k, 512 + 1024*(k+1)).
WX = KT * B + KT * 2 * E * (OC // 2)   # 512 + 8192
WX_CHUNKS = ((0, 4608), (4608, 7680), (7680, 8704))   # xt+k0-3 | k4-6 | k7

# const-pack layout (bf16, [E, CPK]); row 0 additionally carries dense_b
OGT = 0                       # gates.T [E, B]
OOW = OGT + B                 # ow2 [E, 2*L*HC]
OOB = OOW + L * OC            # out_b [E, L] (zeros except core 0)
OGTZ = OOB + L                # gates.T | zeros [E, P]
OGTD = OGTZ + P               # gates.T | gates.T [E, P]
OEBC = OGTD + P               # kron(I_E, ones[HC]) [E, E*HC]
ODB = OEBC + E * HC           # dense_b row (row 0 only) [1, 2*E*HC]
CPK = ODB + 2 * E * HC

_cached = None


def _build():
    from contextlib import ExitStack

    import concourse.tile as tile
    from concourse import bacc, mybir

    F32 = mybir.dt.float32
    BF16 = mybir.dt.bfloat16
    AF = mybir.ActivationFunctionType
    OP = mybir.AluOpType

    nc = bacc.Bacc("TRN2", target_bir_lowering=False, debug=False,
                   num_devices=NCORES)

    wx_d = nc.dram_tensor("wx", [P, WX], BF16, kind="ExternalInput")
    cp_d = nc.dram_tensor("cp", [E, CPK], BF16, kind="ExternalInput")
    out_d = nc.dram_tensor("out", [L, P], F32, kind="ExternalOutput")

    with tile.TileContext(nc) as tc, ExitStack() as ctx:
        consts = ctx.enter_context(tc.tile_pool(name="consts", bufs=1))
        wpool = ctx.enter_context(tc.tile_pool(name="wpool", bufs=1))
        mixp = ctx.enter_context(tc.tile_pool(name="mixp", bufs=2))
        smallp = ctx.enter_context(tc.tile_pool(name="smallp", bufs=1))
        psy = ctx.enter_context(tc.tile_pool(name="psy", bufs=1, space="PSUM"))
        pss = ctx.enter_context(tc.tile_pool(name="pss", bufs=1, space="PSUM"))
        psg = ctx.enter_context(tc.tile_pool(name="psg", bufs=1, space="PSUM"))

        # Sync-ring order = DMA chain service order.  xt rides at the head
        # of the combined wx stream so it shares chunk 0's chain/sem.
        cp_t = consts.tile([E, CPK], BF16)
        nc.sync.dma_start(out=cp_t, in_=cp_d.ap())
        wx_t = wpool.tile([P, WX], BF16)
        for lo, hi in WX_CHUNKS:
            nc.sync.dma_start(
                out=wx_t[:, lo:hi],
                in_=wx_d.ap()[:, lo:hi],
            )
        xt_t = wx_t[:, 0:KT * B].rearrange("p (k b) -> p k b", k=KT)
        w1_t = wx_t[:, KT * B:].rearrange(
            "p (k h e c) -> p k h e c", k=KT, h=2, e=E)

        gt_t = cp_t[:, OGT:OGT + B]
        ow_t = cp_t[:, OOW:OOW + L * OC].rearrange(
            "e (h l c) -> e h l c", h=2, l=L)
        ob_t = cp_t[:, OOB:OOB + L]
        gtz_t = cp_t[:, OGTZ:OGTZ + P]
        gtdup_t = cp_t[:, OGTD:OGTD + P]
        ebc_t = cp_t[:, OEBC:OEBC + E * HC]

        # ---- dense_b fold: psum_y[64h+b, (e,c)] starts at db[e,c] ----
        ones1 = smallp.tile([1, B], BF16)
        nc.vector.memset(ones1[:], 1.0)
        psum_y = psy.tile([P, E, HC], F32)
        for h in range(2):
            nc.tensor.matmul(
                psum_y[h * 64:h * 64 + 64, :, :].rearrange("b e c -> b (e c)"),
                ones1[:],
                cp_t[0:1, ODB + h * E * HC:ODB + (h + 1) * E * HC],
                start=True, stop=False, skip_group_check=True,
            )

        # ---- small matmuls (all bf16, one DMA sem) ----
        # sel_ow^h [64h+b, (l, hc)]
        psum_ow = pss.tile([P, L, HC], F32)
        for h in range(2):
            sl = slice(h * 64, h * 64 + 64)
            nc.tensor.matmul(
                psum_ow[sl, :, :].rearrange("b l c -> b (l c)"),
                gt_t, ow_t[:, h].rearrange("e l c -> e (l c)"),
                start=True, stop=True, skip_group_check=True,
            )
        # Output accumulator [l, p]: seed with sel_ob^T (only core 0
        # carries real ob); the stage-2 transpose accumulates on top.
        psum_o2 = pss.tile([L, P], F32)
        nc.tensor.matmul(psum_o2[:], ob_t, gtz_t,
                         start=True, stop=False, skip_group_check=True)
        # Gate-broadcast table gb[p, (e, hc)] = g[b, e] via one K=8 matmul.
        psum_gb = psg.tile([P, E, HC], F32)
        nc.tensor.matmul(psum_gb[:, :, :].rearrange("p e c -> p (e c)"),
                         gtdup_t, ebc_t, start=True, stop=True)
        gb_t = consts.tile([P, E, HC], F32)
        nc.vector.tensor_copy(gb_t[:], psum_gb[:])

        # Identity for the final PE transpose, built on the idle gpsimd.
        onesq = smallp.tile([P, P], F32)
        nc.gpsimd.memset(onesq[:], 1.0)
        idt_t = consts.tile([P, P], F32)
        nc.gpsimd.affine_select(
            out=idt_t[:], in_=onesq[:], pattern=[[-1, P]],
            compare_op=OP.is_equal, fill=0.0, base=0, channel_multiplier=1,
        )

        # ---- stage 1: y[64h+b, (e, hc)] += x . dense_w[e, oc_half, :] ----
        # k-outer so the PE consumes each w1 chunk as it lands.
        for k in range(KT):
            for h in range(2):
                nc.tensor.matmul(
                    psum_y[h * 64:h * 64 + 64, :, :].rearrange(
                        "b e c -> b (e c)"),
                    xt_t[:, k, :],
                    w1_t[:, k, h].rearrange("p e c -> p (e c)"),
                    start=False,
                    stop=(k == KT - 1),
                    skip_group_check=True,
                )

        # bf16 tree intermediates: the mult reads fp32 PSUM but writes
        # bf16, and the adds then run at 2x DVE throughput.
        prod_t = mixp.tile([P, E, HC], BF16)
        nc.vector.tensor_tensor(
            out=prod_t[:], in0=psum_y[:], in1=gb_t[:], op=OP.mult,
        )
        # contiguous pairwise tree over e (strided reduce is ~2x slower)
        t1 = mixp.tile([P, 4, HC], BF16)
        nc.vector.tensor_add(t1[:], prod_t[:, 0:4, :], prod_t[:, 4:8, :])
        t2 = mixp.tile([P, 2, HC], BF16)
        nc.vector.tensor_add(t2[:], t1[:, 0:2, :], t1[:, 2:4, :])
        t3 = mixp.tile([P, HC], BF16)
        nc.vector.tensor_add(t3[:], t2[:, 0, :], t2[:, 1, :])

        t_t = smallp.tile([P, HC], F32)
        nc.scalar.activation(t_t[:], t3[:], AF.Tanh)

        # ---- stage 2: pre[64h+b, l] = sum_hc t * sel_ow ----
        # NOTE: InstTensorTensorReduce faults TRN2; scalar_tensor_tensor with
        # accum_out (free-dim sum) is the reliable path.
        pre_t = smallp.tile([P, L], F32)
        dump = smallp.tile([P, HC], F32)
        for l in range(L):
            nc.vector.scalar_tensor_tensor(
                out=dump[:],
                in0=psum_ow[:, l, :],
                scalar=1.0,
                in1=t_t[:],
                op0=OP.mult,
                op1=OP.mult,
                accum_out=pre_t[:, l:l + 1],
            )
        # PE transpose [128,2] -> [2,128], accumulating onto the ob seed.
        nc.tensor.matmul(psum_o2[:], pre_t[:], idt_t[:],
                         is_transpose=True, start=False, stop=True,
                         skip_group_check=True)
        o2_t = smallp.tile([L, P], F32)
        nc.vector.tensor_copy(o2_t[:], psum_o2[:])

        nc.sync.dma_start(out=out_d.ap(), in_=o2_t[:])

    nc.compile()
    return nc


def _prep_inputs(X, gates, dense_w, dense_b, out_w, out_b):
    """Host-side layout prep (slice/transpose/cast only) -> per-core maps."""
    import ml_dtypes

    BF = ml_dtypes.bfloat16
    X = np.asarray(X, dtype=np.float32)
    gates = np.asarray(gates, dtype=np.float32)
    dense_w = np.asarray(dense_w, dtype=np.float32)
    dense_b = np.asarray(dense_b, dtype=np.float32)
    out_w = np.asarray(out_w, dtype=np.float32)
    out_b = np.asarray(out_b, dtype=np.float32)

    xcls = X[:, 0, :]                                     # [B, H]
    # xt[i_lo, k, b] = x[b, k*128 + i_lo]
    xt = np.ascontiguousarray(
        xcls.T.reshape(KT, P, B).transpose(1, 0, 2)).astype(BF)
    xt_flat = xt.reshape(P, KT * B)
    gt = np.ascontiguousarray(gates.T)                    # [E, B]
    gtz = np.concatenate([gt, np.zeros_like(gt)], axis=1)  # [E, 128]
    gtdup = np.concatenate([gt, gt], axis=1)               # [E, 128]
    ebc = np.kron(np.eye(E, dtype=np.float32),
                  np.ones((1, HC), dtype=np.float32))      # [E, E*HC]

    in_maps = []
    for c in range(NCORES):
        sl = slice(c * OC, (c + 1) * OC)
        # w1[i_lo, k, h, e, hc] = dense_w[e, c*OC + h*64 + hc, k*128 + i_lo]
        w1 = np.ascontiguousarray(
            dense_w[:, sl, :]                   # [E, OC, H]
            .reshape(E, 2, HC, KT, P)           # [e, h, hc, k, i_lo]
            .transpose(4, 3, 1, 0, 2)           # [i_lo, k, h, e, hc]
        ).astype(BF)
        wx = np.ascontiguousarray(
            np.concatenate([xt_flat, w1.reshape(P, KT * 2 * E * HC)], axis=1))

        # db row: dense_b[e, c*OC + h*64 + hc] laid as (h, e, hc); row 0 only
        dbrow = np.zeros((E, 2 * E * HC), dtype=np.float32)
        dbrow[0] = (dense_b[:, sl].reshape(E, 2, HC)
                    .transpose(1, 0, 2).reshape(-1))

        # ow2[e, (h, l, hc)] = out_w[e, l, c*OC + h*64 + hc]
        ow2 = (out_w[:, :, sl].reshape(E, L, 2, HC)
               .transpose(0, 2, 1, 3).reshape(E, L * OC))
        ob = out_b if c == 0 else np.zeros_like(out_b)
        cp = np.ascontiguousarray(
            np.concatenate([gt, ow2, ob, gtz, gtdup, ebc, dbrow], axis=1)
        ).astype(BF)
        in_maps.append({
            "wx": wx,
            "cp": cp,
        })
    return in_maps


def _run(in_maps, trace=False, tmpdir=None):
    global _cached
    from concourse.bass_utils import run_bass_kernel_spmd

    if _cached is None:
        _cached = _build()
    res = run_bass_kernel_spmd(
        _cached, in_maps, list(range(NCORES)), trace=trace, tmpdir=tmpdir,
    )
    return res


def kernel(X, gates, dense_w, dense_b, out_w, out_b):
    in_maps = _prep_inputs(X, gates, dense_w, dense_b, out_w, out_b)
    res = _run(in_maps)
    acc = np.zeros((B, L), dtype=np.float64)
    for c in range(NCORES):
        part = res.results[c]["out"].astype(np.float64)   # [L, 128]
        acc += part.T.reshape(2, B, L).sum(axis=0)
    return acc.astype(np.float32)


# revision 16
# speedup vs baseline: 1.1778x; 1.0405x over previous
"""Trainium2 Bass kernel for nn_MoEsparseRoutingForClassification.

Reference computation (B=64, S=128, H=1024, E=8, L=2):
    x = X[:, 0, :]                                   # CLS token [B,H]
    y[b,o]   = sum_e g[b,e] * (x[b] . dense_w[e,o,:]) + (g @ dense_b)[b,o]
    t        = tanh(y)
    out[b,l] = sum_e g[b,e] * (t[b] . out_w[e,l,:])  + (g @ out_b)[b,l]

Distribution: the H output dim of the dense layer is sharded 8 ways
(OC=128 per core).  Core c computes y[:, c*OC:(c+1)*OC] (which needs the
full CLS token but only a slice dense_w[:, c_slice, :]), applies tanh,
and contracts its slice against out_w[:, :, c_slice] to produce a
partial [L,128] logit block.  The partials (incl. the out_b bias, fed
only to core 0) sum to the full output on the host.  No cross-core
collective is needed.

Everything that feeds the PE is bf16 (halves HBM traffic, single-pass
matmuls); PSUM accumulation and the DVE mix stay fp32.  rel-err budget
is 2e-2; bf16 rounding lands ~3e-3.

DMA: one ring (sync), ordered so each chain's completion unblocks work
just in time (DMA engines drain one descriptor chain before starting
the next; doorbell->data ~1.5us, dma-complete->sem-visible ~0.5us):
  cp (34 KB const pack: gates/out_w/biases/gate-broadcast consts, one
  sem for everything the small matmuls need) | xt (CLS) | w1 in 3
  chunks (3/3/2 k-tiles) that the PE chases.

PE program order: dense_b fold (K=1 matmul into psum_y, so the bias
needs no DVE add), sel_ow, out_b seed into the [2,128] output psum,
gate-broadcast table (K=8 matmul, replaces a 128-row gc DMA + 8 DVE
builds), then the 16 chunk-chasing stage-1 matmuls.  Post-mix chain:
mult + 3 tree adds + tanh + 2 accum-dots, then a PE transpose
accumulates the [128,2] result onto the out_b seed so the output DMA
is 2 rows x 512 B.

Everything arithmetic runs on device; the host only slices, transposes
(layout prep), and sums the partial outputs.
"""

import sys

import numpy as np

for _p in ("/opt/trn_rl_repo",):
    if _p not in sys.path:
        sys.path.insert(0, _p)

# If the environment sets BASS_TRACE but lacks antenv.axon_hooks (this agent
# image does), run_bass_kernel_spmd would crash on import; pre-seed a no-op
# module so tracing degrades gracefully instead.
try:  # pragma: no cover
    import antenv.axon_hooks  # noqa: F401
except Exception:  # pragma: no cover
    import types as _types

    _m = _types.ModuleType("antenv.axon_hooks")
    _m._hook = None
    _m.set_axon_ntff_profile_hook = lambda h: setattr(_m, "_hook", h)
    _m.get_axon_ntff_profile_hook = lambda: _m._hook
    sys.modules["antenv.axon_hooks"] = _m

B, S, H = 64, 128, 1024
E, L = 8, 2
NCORES = 8
OC = H // NCORES          # dense-output slice per core (128)
HC = OC // 2              # half-slice mapped to a PSUM partition half (64)
KT = H // 128             # contraction tiles
P = 128

# Combined xt|w1 stream chunk boundaries, in bf16 elements per partition:
# xt occupies [0, 512); w1 k-tile k occupies [512 + 1024*k, 512 + 1024*(k+1)).
WX = KT * B + KT * 2 * E * (OC // 2)   # 512 + 8192
WX_CHUNKS = ((0, 4608), (4608, 7680), (7680, 8704))   # xt+k0-3 | k4-6 | k7

# const-pack layout (bf16, [E, CPK]); row 0 additionally carries dense_b
OGT = 0                       # gates.T [E, B]
OOW = OGT + B                 # ow2 [E, 2*L*HC]
OOB = OOW + L * OC            # out_b [E, L] (zeros except core 0)
OGTZ = OOB + L                # gates.T | zeros [E, P]
OGTD = OGTZ + P               # gates.T | gates.T [E, P]
OEBC = OGTD + P               # kron(I_E, ones[HC]) [E, E*HC]
ODB = OEBC + E * HC           # dense_b row (row 0 only) [1, 2*E*HC]
CPK = ODB + 2 * E * HC

_cached = None


def _build():
    from contextlib import ExitStack

    import concourse.tile as tile
    from concourse import bacc, mybir

    F32 = mybir.dt.float32
    BF16 = mybir.dt.bfloat16
    AF = mybir.ActivationFunctionType
    OP = mybir.AluOpType

    nc = bacc.Bacc("TRN2", target_bir_lowering=False, debug=False,
                   num_devices=NCORES)

    wx_d = nc.dram_tensor("wx", [P, WX], BF16, kind="ExternalInput")
    cp_d = nc.dram_tensor("cp", [E, CPK], BF16, kind="ExternalInput")
    out_d = nc.dram_tensor("out", [L, P], F32, kind="ExternalOutput")

    with tile.TileContext(nc) as tc, ExitStack() as ctx:
        consts = ctx.enter_context(tc.tile_pool(name="consts", bufs=1))
        wpool = ctx.enter_context(tc.tile_pool(name="wpool", bufs=1))
        mixp = ctx.enter_context(tc.tile_pool(name="mixp", bufs=2))
        smallp = ctx.enter_context(tc.tile_pool(name="smallp", bufs=1))
        psy = ctx.enter_context(tc.tile_pool(name="psy", bufs=1, space="PSUM"))
        pss = ctx.enter_context(tc.tile_pool(name="pss", bufs=1, space="PSUM"))
        psg = ctx.enter_context(tc.tile_pool(name="psg", bufs=1, space="PSUM"))

        # Sync-ring order = DMA chain service order.  xt rides at the head
        # of the combined wx stream so it shares chunk 0's chain/sem.
        cp_t = consts.tile([E, CPK], BF16)
        nc.sync.dma_start(out=cp_t, in_=cp_d.ap())
        wx_t = wpool.tile([P, WX], BF16)
        for lo, hi in WX_CHUNKS:
            nc.sync.dma_start(
                out=wx_t[:, lo:hi],
                in_=wx_d.ap()[:, lo:hi],
            )
        xt_t = wx_t[:, 0:KT * B].rearrange("p (k b) -> p k b", k=KT)
        w1_t = wx_t[:, KT * B:].rearrange(
            "p (k h e c) -> p k h e c", k=KT, h=2, e=E)

        gt_t = cp_t[:, OGT:OGT + B]
        ow_t = cp_t[:, OOW:OOW + L * OC].rearrange(
            "e (h l c) -> e h l c", h=2, l=L)
        ob_t = cp_t[:, OOB:OOB + L]
        gtz_t = cp_t[:, OGTZ:OGTZ + P]
        gtdup_t = cp_t[:, OGTD:OGTD + P]
        ebc_t = cp_t[:, OEBC:OEBC + E * HC]

        # ---- dense_b fold: psum_y[64h+b, (e,c)] starts at db[e,c] ----
        ones1 = smallp.tile([1, B], BF16)
        nc.vector.memset(ones1[:], 1.0)
        psum_y = psy.tile([P, E, HC], F32)
        for h in range(2):
            nc.tensor.matmul(
                psum_y[h * 64:h * 64 + 64, :, :].rearrange("b e c -> b (e c)"),
                ones1[:],
                cp_t[0:1, ODB + h * E * HC:ODB + (h + 1) * E * HC],
                start=True, stop=False, skip_group_check=True,
            )

        # ---- small matmuls (all bf16, one DMA sem) ----
        # sel_ow^h [64h+b, (l, hc)]
        psum_ow = pss.tile([P, L, HC], F32)
        for h in range(2):
            sl = slice(h * 64, h * 64 + 64)
            nc.tensor.matmul(
                psum_ow[sl, :, :].rearrange("b l c -> b (l c)"),
                gt_t, ow_t[:, h].rearrange("e l c -> e (l c)"),
                start=True, stop=True, skip_group_check=True,
            )
        # Output accumulator [l, p]: seed with sel_ob^T (only core 0
        # carries real ob); the stage-2 transpose accumulates on top.
        psum_o2 = pss.tile([L, P], F32)
        nc.tensor.matmul(psum_o2[:], ob_t, gtz_t,
                         start=True, stop=False, skip_group_check=True)
        # Gate-broadcast table gb[p, (e, hc)] = g[b, e] via one K=8 matmul.
        psum_gb = psg.tile([P, E, HC], F32)
        nc.tensor.matmul(psum_gb[:, :, :].rearrange("p e c -> p (e c)"),
                         gtdup_t, ebc_t, start=True, stop=True)
        gb_t = consts.tile([P, E, HC], F32)
        nc.vector.tensor_copy(gb_t[:], psum_gb[:])

        # Identity for the final PE transpose, built on the idle gpsimd.
        onesq = smallp.tile([P, P], F32)
        nc.gpsimd.memset(onesq[:], 1.0)
        idt_t = consts.tile([P, P], F32)
        nc.gpsimd.affine_select(
            out=idt_t[:], in_=onesq[:], pattern=[[-1, P]],
            compare_op=OP.is_equal, fill=0.0, base=0, channel_multiplier=1,
        )

        # ---- stage 1: y[64h+b, (e, hc)] += x . dense_w[e, oc_half, :] ----
        # k-outer so the PE consumes each w1 chunk as it lands.
        # NOTE: splitting the last k-tile into e-halves (partial-width
        # stop matmuls) hard-faults the PE (NRT_EXEC_UNIT_UNRECOVERABLE);
        # keep full-width accumulation.
        for k in range(KT):
            for h in range(2):
                nc.tensor.matmul(
                    psum_y[h * 64:h * 64 + 64, :, :].rearrange(
                        "b e c -> b (e c)"),
                    xt_t[:, k, :],
                    w1_t[:, k, h].rearrange("p e c -> p (e c)"),
                    start=False,
                    stop=(k == KT - 1),
                    skip_group_check=True,
                )

        # bf16 tree intermediates: the mult reads fp32 PSUM but writes
        # bf16, and the adds then run at 2x DVE throughput.
        prod_t = mixp.tile([P, E, HC], BF16)
        nc.vector.tensor_tensor(
            out=prod_t[:], in0=psum_y[:], in1=gb_t[:], op=OP.mult,
        )
        # contiguous pairwise tree over e (strided reduce is ~2x slower)
        t1 = mixp.tile([P, 4, HC], BF16)
        nc.vector.tensor_add(t1[:], prod_t[:, 0:4, :], prod_t[:, 4:8, :])
        t2 = mixp.tile([P, 2, HC], BF16)
        nc.vector.tensor_add(t2[:], t1[:, 0:2, :], t1[:, 2:4, :])
        t3 = mixp.tile([P, HC], BF16)
        nc.vector.tensor_add(t3[:], t2[:, 0, :], t2[:, 1, :])

        t_t = smallp.tile([P, HC], F32)
        nc.scalar.activation(t_t[:], t3[:], AF.Tanh)

        # ---- stage 2: pre[64h+b, l] = sum_hc t * sel_ow ----
        # NOTE: InstTensorTensorReduce faults TRN2; scalar_tensor_tensor with
        # accum_out (free-dim sum) is the reliable path.
        pre_t = smallp.tile([P, L], F32)
        dump = smallp.tile([P, HC], F32)
        for l in range(L):
            nc.vector.scalar_tensor_tensor(
                out=dump[:],
                in0=psum_ow[:, l, :],
                scalar=1.0,
                in1=t_t[:],
                op0=OP.mult,
                op1=OP.mult,
                accum_out=pre_t[:, l:l + 1],
            )
        # PE transpose [128,2] -> [2,128], accumulating onto the ob seed.
        nc.tensor.matmul(psum_o2[:], pre_t[:], idt_t[:],
                         is_transpose=True, start=False, stop=True,
                         skip_group_check=True)
        o2_t = smallp.tile([L, P], F32)
        nc.vector.tensor_copy(o2_t[:], psum_o2[:])

        nc.sync.dma_start(out=out_d.ap(), in_=o2_t[:])

    nc.compile()
    return nc


def _prep_inputs(X, gates, dense_w, dense_b, out_w, out_b):
    """Host-side layout prep (slice/transpose/cast only) -> per-core maps."""
    import ml_dtypes

    BF = ml_dtypes.bfloat16
    X = np.asarray(X, dtype=np.float32)
    gates = np.asarray(gates, dtype=np.float32)
    dense_w = np.asarray(dense_w, dtype=np.float32)
    dense_b = np.asarray(dense_b, dtype=np.float32)
    out_w = np.asarray(out_w, dtype=np.float32)
    out_b = np.asarray(out_b, dtype=np.float32)

    xcls = X[:, 0, :]                                     # [B, H]
    # xt[i_lo, k, b] = x[b, k*128 + i_lo]
    xt = np.ascontiguousarray(
        xcls.T.reshape(KT, P, B).transpose(1, 0, 2)).astype(BF)
    xt_flat = xt.reshape(P, KT * B)
    gt = np.ascontiguousarray(gates.T)                    # [E, B]
    gtz = np.concatenate([gt, np.zeros_like(gt)], axis=1)  # [E, 128]
    gtdup = np.concatenate([gt, gt], axis=1)               # [E, 128]
    ebc = np.kron(np.eye(E, dtype=np.float32),
                  np.ones((1, HC), dtype=np.float32))      # [E, E*HC]

    in_maps = []
    for c in range(NCORES):
        sl = slice(c * OC, (c + 1) * OC)
        # w1[i_lo, k, h, e, hc] = dense_w[e, c*OC + h*64 + hc, k*128 + i_lo]
        w1 = np.ascontiguousarray(
            dense_w[:, sl, :]                   # [E, OC, H]
            .reshape(E, 2, HC, KT, P)           # [e, h, hc, k, i_lo]
            .transpose(4, 3, 1, 0, 2)           # [i_lo, k, h, e, hc]
        ).astype(BF)
        wx = np.ascontiguousarray(
            np.concatenate([xt_flat, w1.reshape(P, KT * 2 * E * HC)], axis=1))

        # db row: dense_b[e, c*OC + h*64 + hc] laid as (h, e, hc); row 0 only
        dbrow = np.zeros((E, 2 * E * HC), dtype=np.float32)
        dbrow[0] = (dense_b[:, sl].reshape(E, 2, HC)
                    .transpose(1, 0, 2).reshape(-1))

        # ow2[e, (h, l, hc)] = out_w[e, l, c*OC + h*64 + hc]
        ow2 = (out_w[:, :, sl].reshape(E, L, 2, HC)
               .transpose(0, 2, 1, 3).reshape(E, L * OC))
        ob = out_b if c == 0 else np.zeros_like(out_b)
        cp = np.ascontiguousarray(
            np.concatenate([gt, ow2, ob, gtz, gtdup, ebc, dbrow], axis=1)
        ).astype(BF)
        in_maps.append({
            "wx": wx,
            "cp": cp,
        })
    return in_maps


def _run(in_maps, trace=False, tmpdir=None):
    global _cached
    from concourse.bass_utils import run_bass_kernel_spmd

    if _cached is None:
        _cached = _build()
    res = run_bass_kernel_spmd(
        _cached, in_maps, list(range(NCORES)), trace=trace, tmpdir=tmpdir,
    )
    return res


def kernel(X, gates, dense_w, dense_b, out_w, out_b):
    in_maps = _prep_inputs(X, gates, dense_w, dense_b, out_w, out_b)
    res = _run(in_maps)
    acc = np.zeros((B, L), dtype=np.float64)
    for c in range(NCORES):
        part = res.results[c]["out"].astype(np.float64)   # [L, 128]
        acc += part.T.reshape(2, B, L).sum(axis=0)
    return acc.astype(np.float32)
